# revision 10
# baseline (speedup 1.0000x reference)
"""Additive (Bahdanau) attention Trainium2 kernel.

reference:
    q = query @ Wq_w.T + Wq_b                       # [B, D]
    k = keys @ Wk_w.T + Wk_b                        # [B, S, D]
    scores = einsum('bsd,d->bs', tanh(q[:,None,:] + k), v_w) + v_b
    attn = softmax(scores, -1)                      # [B, S]
    context = einsum('bs,bsd->bd', attn, keys)      # [B, D]
    returns (context, attn)

B=32, S=2048, D=1024. Data-parallel over batch: 8 cores x 4 batches each.
v_b is dropped (softmax is shift-invariant; raw scores are not returned).

Per-core pipeline (matmuls in fp32r -> full PE rate, ~1.5e-4 rel err):
  proj:  k_projT[dout,s] = sum_din Wk_wT[din,dout] * keysT[din,s]   (PE)
         computed in two s-halves of 1024 for keysT DMA double-buffering
  tanh:  t = tanh(k_projT + (q_proj + Wq_b + Wk_b)[dout])           (ACT, fused)
  vdot:  scores[1, s] += v[dout_tile]^T @ t[dout_tile]              (PE)
  softmax on [1, 2048]  (DVE max / ACT exp with fused sum / DVE recip)
  attnT via small DRAM roundtrip -> [128, 16]
  ctx:   context += attnT[:, st]^T @ keys[st]   (PE, interleaved mid-proj
         of the next batch to hide softmax latency and smooth DMA load)
"""
import sys

if "/opt/trn_rl_repo" not in sys.path:
    sys.path.insert(0, "/opt/trn_rl_repo")

import numpy as np

import concourse.bacc as bacc
import concourse.mybir as mybir
import concourse.tile as tile

B, S, D = 32, 2048, 1024
NCORES = 8
BPC = B // NCORES          # batches per core
F32 = mybir.dt.float32
DTR = mybir.dt.float32r
TANH = mybir.ActivationFunctionType.Tanh
EXP = mybir.ActivationFunctionType.Exp
IDENT = mybir.ActivationFunctionType.Identity

NDT = D // 128             # 8 dout/din tiles
NST = S // 128             # 16 s tiles (context)
SH = S // 2                # 1024: s-half width
NH = 2                     # halves
NCH = SH // 512            # 2 chunks per half
NDC = D // 512             # 2 d chunks (context out)


def r(ap):
    """bitcast to fp32r: full-rate single-pass fp32 matmul on the PE"""
    return ap.bitcast(DTR)


def build_nc(reps=1, use_for_i=False, nbatch=BPC, do_softmax=True, do_ctx=True):
    nc = bacc.Bacc(None, target_bir_lowering=False)

    kT_d = nc.declare_dram_parameter("kT", [BPC, D, S], DTR, isOutput=False)
    kN_d = nc.declare_dram_parameter("kN", [BPC, S, D], DTR, isOutput=False)
    wk_d = nc.declare_dram_parameter("wk", [D, D], DTR, isOutput=False)   # [din,dout]
    wq_d = nc.declare_dram_parameter("wq", [D, D], DTR, isOutput=False)   # [din,dout]
    qT_d = nc.declare_dram_parameter("qT", [D, BPC], DTR, isOutput=False)
    bqk_d = nc.declare_dram_parameter("bqk", [128, NDT], F32, isOutput=False)
    vw_d = nc.declare_dram_parameter("vw", [128, NDT], DTR, isOutput=False)
    ctx_d = nc.declare_dram_parameter("ctx", [BPC, D], F32, isOutput=True)
    attn_d = nc.declare_dram_parameter("attn", [BPC, S], F32, isOutput=True)
    sc_d = nc.dram_tensor("sc", [BPC, S], F32)

    # [b, half, din_tile, 128, SH]
    kT_r = kT_d.rearrange("b (t p) (h s) -> b h t p s", p=128, h=NH)
    kN_r = kN_d.rearrange("b (t p) d -> b t p d", p=128)    # [BPC, NST, 128, D]
    wk_r = wk_d.rearrange("(t p) d -> t p d", p=128)        # [NDT, 128, D]
    wq_r = wq_d.rearrange("(t p) d -> t p d", p=128)
    qT_r = qT_d.rearrange("(t p) b -> t p b", p=128)        # [NDT, 128, BPC]

    with tile.TileContext(nc) as tc:
        with (
            tc.tile_pool(name="wc", bufs=1) as wc,           # wk resident 32KB/p
            tc.tile_pool(name="kt", bufs=16) as ktp,         # keysT halves 64KB/p
            tc.tile_pool(name="kn", bufs=4) as knp,          # keys natural 16KB/p
            tc.tile_pool(name="tt", bufs=3) as ttp,          # tanh tiles 12KB/p
            tc.tile_pool(name="sm", bufs=2) as smp,          # softmax + outs
            tc.tile_pool(name="dr", bufs=2, space="DRAM") as drp,  # attn bounce
            tc.tile_pool(name="mi", bufs=1) as mip,          # small constants
            tc.tile_pool(name="pp", bufs=3, space="PSUM") as ppp,  # proj [128,512]
            tc.tile_pool(name="ac", bufs=3, space="PSUM") as acp,  # scores banks
            tc.tile_pool(name="cc", bufs=2, space="PSUM") as ccp,  # ctx banks
        ):
            # ---------------- constants ----------------
            wk_sb = wc.tile([128, NDT, D], DTR, tag="wk")
            for t in range(NDT):
                nc.sync.dma_start(wk_sb[:, t, :], wk_r[t])
            qT_sb = mip.tile([128, NDT, BPC], DTR, tag="qT")
            for t in range(NDT):
                nc.sync.dma_start(qT_sb[:, t, :], qT_r[t])
            bqk_sb = mip.tile([128, NDT], F32, tag="bqk")
            nc.sync.dma_start(bqk_sb[:], bqk_d[:])
            vw_sb = mip.tile([128, NDT], DTR, tag="vw")
            nc.sync.dma_start(vw_sb[:], vw_d[:])
            qkb_sb = mip.tile([128, NDT, BPC], F32, tag="qkb")

            # ------------- q projection (once) -------------
            wq_tiles = []
            for t in range(NDT):
                wqt = ktp.tile([128, SH], DTR, tag="kt")
                nc.sync.dma_start(wqt[:], wq_r[t])
                wq_tiles.append(wqt)
            qp = ppp.tile([128, NDT * BPC], F32, tag="pp")
            for t in range(NDT):
                for din in range(NDT):
                    nc.tensor.matmul(
                        qp[:, t * BPC:(t + 1) * BPC],
                        r(wq_tiles[din][:, t * 128:(t + 1) * 128]),
                        r(qT_sb[:, din, :]),
                        start=(din == 0), stop=(din == NDT - 1),
                    )
            for t in range(NDT):
                nc.scalar.activation(qkb_sb[:, t, :], qp[:, t * BPC:(t + 1) * BPC],
                                     IDENT, bias=bqk_sb[:, t:t + 1])

            st_ = _State()
            halves = [(b, h) for b in range(nbatch) for h in range(NH)]

            def emit_kt_dmas(b, h):
                for din in range(NDT):
                    ktt = ktp.tile([128, SH], DTR, tag="kt")
                    nc.sync.dma_start(ktt[:], kT_r[b, h, din])
                    st_.kt[(b, h, din)] = ktt

            def emit_ctx_portion(n):
                """emit next n s-tiles of the pending context matmul"""
                if st_.ctx_b is None:
                    return
                b = st_.ctx_b
                while n > 0 and st_.ctx_st < NST:
                    stt = st_.ctx_st
                    knt = knp.tile([128, D], DTR, tag="kn")
                    nc.sync.dma_start(knt[:], kN_r[b, stt])
                    for dc in range(NDC):
                        nc.tensor.matmul(
                            st_.cc_t[dc][0:1, :],
                            r(st_.attnT[:, stt:stt + 1]),
                            r(knt[:, dc * 512:(dc + 1) * 512]),
                            start=(stt == 0), stop=(stt == NST - 1),
                        )
                    st_.ctx_st += 1
                    n -= 1
                if st_.ctx_st == NST:
                    ctx_sb = smp.tile([1, D], F32, tag="ctx")
                    for dc in range(NDC):
                        nc.scalar.copy(ctx_sb[:, dc * 512:(dc + 1) * 512],
                                       st_.cc_t[dc][0:1, :])
                    nc.sync.dma_start(ctx_d[b:b + 1, :], ctx_sb[:])
                    st_.ctx_b = None

            for rep in range(reps):
                emit_kt_dmas(0, 0)
                emit_kt_dmas(0, 1)
                for idx, (b, h) in enumerate(halves):
                    if idx + 2 < len(halves):
                        emit_kt_dmas(*halves[idx + 2])
                    st_.acc_v = [acp.tile([128, 512], F32, tag="ac",
                                          name=f"acc{i}") for i in range(NCH)]
                    if h == 0:
                        st_.scores = smp.tile([1, S], F32, tag="scores")
                    t_prev = None
                    for dt_ in range(NDT):
                        # ---- projection group for (b, h, dout=dt_) ----
                        tt = ttp.tile([128, SH], DTR, tag="tt")
                        pp_c = [ppp.tile([128, 512], F32, tag="pp", name=f"pp{c}")
                                for c in range(NCH)]
                        for din in range(NDT):
                            wblk = r(wk_sb[:, din, dt_ * 128:(dt_ + 1) * 128])
                            for c in range(NCH):
                                nc.tensor.matmul(
                                    pp_c[c][:], wblk,
                                    r(st_.kt[(b, h, din)][:, c * 512:(c + 1) * 512]),
                                    start=(din == 0), stop=(din == NDT - 1),
                                )
                        for c in range(NCH):
                            nc.scalar.activation(
                                tt[:, c * 512:(c + 1) * 512], pp_c[c][:],
                                TANH, bias=qkb_sb[:, dt_, b:b + 1])
                        # ---- vdot for previous dout (1 behind for ACT slack) --
                        if t_prev is not None:
                            _vdot(nc, st_, vw_sb, t_prev, dt_ - 1, h)
                        t_prev = tt
                        # ---- interleave prev batch's ctx (2 tiles / dout) ----
                        if (h, dt_) >= (0, 3):
                            emit_ctx_portion(2)
                    _vdot(nc, st_, vw_sb, t_prev, NDT - 1, h)
                    # drain this half's score chunks (frees the ac banks)
                    scores = st_.scores
                    for c in range(NCH):
                        g = 2 * h + c
                        nc.scalar.copy(scores[:, g * 512:(g + 1) * 512],
                                       st_.acc_v[c][0:1, :])
                    if h == 0:
                        continue
                    # release this batch's kt keys
                    for k in [k for k in st_.kt if k[0] == b]:
                        del st_.kt[k]

                    import os as _os
                    _lvl = int(_os.environ.get("SM_LVL", "9"))
                    if not do_softmax:
                        nc.sync.dma_start(attn_d[b:b + 1, :], scores[:])
                        continue
                    mx = smp.tile([1, 1], F32, tag="mx")
                    nc.vector.reduce_max(mx[:], scores[:],
                                         axis=mybir.AxisListType.X)
                    nmx = smp.tile([1, 1], F32, tag="nmx")
                    nc.scalar.mul(nmx[:], mx[:], -1.0)
                    if _lvl < 2:
                        nc.sync.dma_start(attn_d[b:b + 1, :], scores[:])
                        continue
                    p_sb = smp.tile([1, S], F32, tag="p")
                    ssum = smp.tile([1, 1], F32, tag="ssum")
                    nc.scalar.activation(p_sb[:], scores[:], EXP,
                                         bias=nmx[0:1, 0:1],
                                         accum_out=ssum[:] if _lvl >= 3 else None)
                    if _lvl < 4:
                        nc.sync.dma_start(attn_d[b:b + 1, :], p_sb[:])
                        continue
                    rs = smp.tile([1, 1], F32, tag="rs")
                    nc.vector.reciprocal(rs[:], ssum[:])
                    if _lvl >= 5:
                        nc.vector.tensor_scalar_mul(p_sb[:], p_sb[:], rs[0:1, 0:1])
                    nc.sync.dma_start(attn_d[b:b + 1, :], p_sb[:])
                    if _lvl < 6:
                        continue
                    # transpose bounce: [1, S] -> DRAM -> [128, NST]
                    nc.sync.dma_start(sc_d[b:b + 1, :], p_sb[:])
                    attnT = smp.tile([128, NST], DTR, tag="attnT")
                    nc.sync.dma_start(attnT[:],
                                      r(sc_d[b].rearrange("(f p) -> p f", p=128)))
                    st_.attnT = attnT
                    if not do_ctx:
                        continue
                    st_.ctx_b = b
                    st_.ctx_st = 0
                    st_.cc_t = [ccp.tile([128, 512], F32, tag="cc",
                                         name=f"cc{i}") for i in range(NDC)]
                # drain last batch's context
                emit_ctx_portion(NST)
    nc.finalize()
    return nc


class _State:
    def __init__(self):
        self.kt = {}
        self.acc_v = None
        self.scores = None
        self.attnT = None
        self.ctx_b = None
        self.ctx_st = 0
        self.cc_t = None


def _vdot(nc, st_, vw_sb, tt, dt_, h):
    """scores chunks (2h, 2h+1) += v[dt_]^T @ t[dt_] for this half"""
    for c in range(NCH):
        nc.tensor.matmul(
            st_.acc_v[c][0:1, :], r(vw_sb[:, dt_:dt_ + 1]),
            r(tt[:, c * 512:(c + 1) * 512]),
            start=(dt_ == 0), stop=(dt_ == NDT - 1),
        )


_NC_CACHE = {}


def _get_nc(reps=1):
    if reps not in _NC_CACHE:
        _NC_CACHE[reps] = build_nc(reps)
    return _NC_CACHE[reps]


def _prep_core_inputs(query, keys, Wq_w, Wq_b, Wk_w, Wk_b, v_w, v_b):
    """host-side shard + layout prep; returns list of per-core input maps"""
    query = np.asarray(query, dtype=np.float32)
    keys = np.ascontiguousarray(np.asarray(keys, dtype=np.float32))
    wkT = np.ascontiguousarray(np.asarray(Wk_w, dtype=np.float32).T)  # [din,dout]
    wqT = np.ascontiguousarray(np.asarray(Wq_w, dtype=np.float32).T)
    bqk = (np.asarray(Wq_b, dtype=np.float32)
           + np.asarray(Wk_b, dtype=np.float32))                       # [D]
    bqk_rs = np.ascontiguousarray(bqk.reshape(NDT, 128).T)             # [128, NDT]
    vw_rs = np.ascontiguousarray(
        np.asarray(v_w, dtype=np.float32).reshape(NDT, 128).T)         # [128, NDT]

    in_maps = []
    for c in range(NCORES):
        b0 = c * BPC
        kb = keys[b0:b0 + BPC]                                         # [BPC, S, D]
        in_maps.append({
            "kT": np.ascontiguousarray(kb.transpose(0, 2, 1)),         # [BPC, D, S]
            "kN": kb,
            "wk": wkT,
            "wq": wqT,
            "qT": np.ascontiguousarray(query[b0:b0 + BPC].T),          # [D, BPC]
            "bqk": bqk_rs,
            "vw": vw_rs,
        })
    return in_maps


def kernel(query, keys, Wq_w, Wq_b, Wk_w, Wk_b, v_w, v_b):
    from concourse.bass_utils import run_bass_kernel_spmd

    nc = _get_nc()
    in_maps = _prep_core_inputs(query, keys, Wq_w, Wq_b, Wk_w, Wk_b, v_w, v_b)
    res = run_bass_kernel_spmd(nc, in_maps, list(range(NCORES)))
    ctx = np.concatenate([res.results[c]["ctx"] for c in range(NCORES)], axis=0)
    attn = np.concatenate([res.results[c]["attn"] for c in range(NCORES)], axis=0)
    return ctx, attn
